# revision 32
# baseline (speedup 1.0000x reference)
"""GQA attention block (16 q heads / 2 kv heads, RoPE, causal) on 8 TRN2 NeuronCores.

Strategy: tensor-parallel over heads. Each core owns 2 q heads + the matching
kv head (kv heads replicated over 4-core groups), computes its partial o_proj
output over the full sequence, and the host sums the 8 partials. All cores run
the identical graph; only the input *data* differs per core (SPMD-safe).

Dataflow (everything "transposed" so no on-chip transpose of activations is
ever needed):
  - host passes x^T (bf16) pre-blocked per 512-seq window so every DMA is one
    contiguous read; weights are host-rearranged to [128, chunk*cols] likewise
  - projections compute Q^T (2 heads), K^T and V per window with the weight
    chunk stationary and x^T streaming; K/V are computed by every core
    (replicating them via a 4-core AllGather measured slower than recompute)
  - RoPE head-dim is host-permuted so rotate-half partners sit on adjacent
    partitions: the swap is a DVE within-quadrant stream_shuffle; bias adds
    ride a DVE tensor_scalar_add out of PSUM
  - scores are computed transposed: S^T[key, q] = K^T_chunk.T @ Q^T
  - softmax without max-subtraction, shifted: P = exp(s*scale - 6) on ACT,
    written straight to bf16 (|scaled scores| < 6 so this is exact enough and
    the shift cancels in the ratio)
  - causal masking multiplies the diagonal-band chunks with 0/1 masks (DVE)
  - denominator: fp16 accumulation of P^T groups on DVE (fp16 keeps the 2x
    perf mode; bf16 would too but fp16 halves the rounding error), a
    ones-vector matmul partition-reduce, reciprocal_approx_fast, and a
    gpsimd partition_broadcast; the scale folds into the out^T -> SBUF copy
  - PV accumulates out^T[d, q] with V (natural layout, via PE transpose)
    stationary and P^T streaming
  - o_proj uses out^T slices as the stationary operand directly; o_proj for
    window j-1 is interleaved between the two heads of window j to keep PE fed

Measured: 386 us HW exec (neuron-profile), rel l2 err 5.3e-3 vs the fp32
reference. PE is the bottleneck (median matmul spacing at the 216 ns N=512
hardware floor; ~33 us total PE idle).
"""

import os
import sys

for _p in ("/opt/trn_rl_repo",):
    if os.path.isdir(_p) and _p not in sys.path:
        sys.path.append(_p)

import numpy as np
import ml_dtypes

FP16 = np.float16
BF16 = ml_dtypes.bfloat16

# ---- problem constants (hardcoded per harness contract) ----
S = 4096          # sequence length
H = 2048          # hidden
DH = 128          # head dim
N_CORES = 8
HC = H // 128     # 16 hidden chunks
W = 512           # q-window width
NW = S // W       # 8 windows
SQ = S // 4       # sequence quarter (per-core K/V share)
SCALE = 1.0 / float(np.sqrt(DH))
EXP_SHIFT = -6.0

_CACHE = {}


def _build():
    import concourse.bacc as bacc
    import concourse.mybir as mybir
    import concourse.tile as tile
    from concourse.masks import make_identity

    dt = mybir.dt
    AF = mybir.ActivationFunctionType

    nc = bacc.Bacc("TRN2", target_bir_lowering=False, debug=False,
                   num_devices=N_CORES)

    xt = nc.dram_tensor("xt", [NW, 128, HC * W], dt.bfloat16, kind="ExternalInput")
    wq = nc.dram_tensor("wq", [128, HC * 2 * DH], dt.bfloat16, kind="ExternalInput")
    wk = nc.dram_tensor("wk", [128, HC * DH], dt.bfloat16, kind="ExternalInput")
    wv = nc.dram_tensor("wv", [128, HC * DH], dt.bfloat16, kind="ExternalInput")
    wo = nc.dram_tensor("wo", [128, 2 * H], dt.bfloat16, kind="ExternalInput")
    bqd = nc.dram_tensor("bq", [128, 2], dt.float32, kind="ExternalInput")
    bkvd = nc.dram_tensor("bkv", [128, 2], dt.float32, kind="ExternalInput")
    cosd = nc.dram_tensor("cost", [128, S], dt.bfloat16, kind="ExternalInput")
    sind = nc.dram_tensor("sins", [128, S], dt.bfloat16, kind="ExternalInput")
    mskd = nc.dram_tensor("msk", [128, 4 * W], dt.bfloat16, kind="ExternalInput")
    out = nc.dram_tensor("out", [S, H], dt.float32, kind="ExternalOutput")

    with tile.TileContext(nc) as tc:
        with (
            tc.tile_pool(name="const", bufs=1) as constp,
            tc.tile_pool(name="xtp", bufs=2) as xtp,
            tc.tile_pool(name="proj", bufs=1) as projp,
            tc.tile_pool(name="ptp", bufs=8) as ptp,
            tc.tile_pool(name="work", bufs=2) as workp,
            tc.tile_pool(name="otsp", bufs=5) as otsp,
            tc.tile_pool(name="obp", bufs=2) as obp,
            tc.tile_pool(name="pp", bufs=2, space="PSUM") as pp,
            tc.tile_pool(name="pqk", bufs=2, space="PSUM") as pqk,
            tc.tile_pool(name="ppv", bufs=2, space="PSUM") as ppv,
        ):
            # ---------- constants into SBUF ----------
            wq_sb = constp.tile([128, HC * 2 * DH], dt.bfloat16, tag="wq")
            wk_sb = constp.tile([128, HC * DH], dt.bfloat16, tag="wk")
            wv_sb = constp.tile([128, HC * DH], dt.bfloat16, tag="wv")
            wo_sb = constp.tile([128, 2 * H], dt.bfloat16, tag="wo")
            bq_sb = constp.tile([128, 2], dt.float32, tag="bq")
            bkv_sb = constp.tile([128, 2], dt.float32, tag="bkv")
            cos_sb = constp.tile([128, S], dt.bfloat16, tag="cos")
            sin_sb = constp.tile([128, S], dt.bfloat16, tag="sin")
            msk_sb = constp.tile([128, 4 * W], dt.bfloat16, tag="msk")
            ones_sb = constp.tile([128, 1], dt.float16, tag="ones")
            ident = constp.tile([128, 128], dt.bfloat16, tag="ident")
            negC = constp.tile([128, 1], dt.float32, tag="negC")

            nc.sync.dma_start(wq_sb[:], wq[:, :])
            nc.gpsimd.dma_start(wk_sb[:], wk[:, :])
            nc.gpsimd.dma_start(wv_sb[:], wv[:, :])
            nc.gpsimd.dma_start(bq_sb[:], bqd[:, :])
            nc.gpsimd.dma_start(bkv_sb[:], bkvd[:, :])
            nc.gpsimd.dma_start(cos_sb[:], cosd[:, :])
            nc.gpsimd.dma_start(sin_sb[:], sind[:, :])
            nc.gpsimd.dma_start(msk_sb[:], mskd[:, :])
            nc.gpsimd.dma_start(wo_sb[:], wo[:, :])
            nc.gpsimd.memset(ones_sb[:], 1.0)
            nc.gpsimd.memset(negC[:], EXP_SHIFT)
            make_identity(nc, ident[:])

            qt_sb = projp.tile([128, 2 * S], dt.bfloat16, tag="qt")
            kt_q = [projp.tile([128, SQ], dt.bfloat16, tag=f"ktq{r}",
                               name=f"ktq{r}") for r in range(4)]
            vn_q = [projp.tile([128, SQ], dt.bfloat16, tag=f"vnq{r}",
                               name=f"vnq{r}") for r in range(4)]

            def kt_chunk(k):
                return kt_q[k // 8][:, (k % 8) * 128:(k % 8 + 1) * 128]

            def vn_chunk(k):
                return vn_q[k // 8][:, (k % 8) * 128:(k % 8 + 1) * 128]

            shuffle_mask = [i ^ 1 for i in range(32)]

            def rope_store(ps, bias, dest_slc, cslc, sslc):
                t0 = workp.tile([128, W], dt.bfloat16, tag="rope0")
                nc.vector.tensor_scalar_add(t0[:], ps[:], bias)
                tsw = workp.tile([128, W], dt.bfloat16, tag="ropesw")
                nc.vector.stream_shuffle(tsw[:], t0[:], mask=shuffle_mask)
                t1 = workp.tile([128, W], dt.bfloat16, tag="rope1")
                nc.vector.tensor_mul(t1[:], t0[:], cslc)
                t2 = workp.tile([128, W], dt.bfloat16, tag="rope2")
                nc.vector.tensor_mul(t2[:], tsw[:], sslc)
                nc.vector.tensor_add(dest_slc, t1[:], t2[:])

            # ---------- phase 1: Q^T (2 heads) + K^T + V projections ----------
            for sb in range(NW):
                xb = xtp.tile([128, HC * W], dt.bfloat16, tag="xtb")
                nc.sync.dma_start(xb[:], xt[sb, :, :])
                targets = [
                    ("rope", lambda h: wq_sb[:, h * 256:h * 256 + 128],
                     bq_sb[:, 0:1], qt_sb, 0, cos_sb, sin_sb),
                    ("rope", lambda h: wq_sb[:, h * 256 + 128:h * 256 + 256],
                     bq_sb[:, 1:2], qt_sb, S, cos_sb, sin_sb),
                    ("rope", lambda h: wk_sb[:, h * 128:(h + 1) * 128],
                     bkv_sb[:, 0:1], kt_q[sb // 2], -(sb // 2) * 2 * W,
                     cos_sb, sin_sb),
                    ("vnat", lambda h: wv_sb[:, h * 128:(h + 1) * 128],
                     bkv_sb[:, 1:2], vn_q[sb // 2], 0, None, None),
                ]
                for kind, wslc, bias, dest, doff, ctab, stab in targets:
                    ps = pp.tile([128, W], dt.float32, tag="pp_ps")
                    for h in range(HC):
                        nc.tensor.matmul(
                            ps[:], wslc(h), xb[:, h * W:(h + 1) * W],
                            start=(h == 0), stop=(h == HC - 1))
                    if kind == "rope":
                        rope_store(ps, bias,
                                   dest[:, doff + sb * W: doff + (sb + 1) * W],
                                   ctab[:, sb * W:(sb + 1) * W],
                                   stab[:, sb * W:(sb + 1) * W])
                    else:
                        t0 = workp.tile([128, W], dt.bfloat16, tag="vstage")
                        nc.vector.tensor_scalar_add(t0[:], ps[:], bias)
                        for i in range(W // 128):
                            tp = ppv.tile([128, 128], dt.bfloat16, tag="ppv_ps")
                            nc.tensor.transpose(
                                tp[:], t0[:, i * 128:(i + 1) * 128], ident[:])
                            nc.vector.tensor_copy(
                                dest[:, ((sb % 2) * 4 + i) * 128:
                                     ((sb % 2) * 4 + i + 1) * 128],
                                tp[:])

            # ---------- phase 2: attention, with o_proj(j-1) interleaved ----------
            def attn_head(a, j, spreader=None):
                nkc = 4 * j + 4
                qslc = qt_sb[:, a * S + j * W: a * S + (j + 1) * W]
                ot = ppv.tile([128, W], dt.float32, tag="ppv_ps")
                dacc = workp.tile([128, 2 * W], dt.float16, tag="dacc")
                for g in range(nkc // 2):
                    ps = pqk.tile([128, 2 * W], dt.float32, tag="qk_ps")
                    ptg = ptp.tile([128, 2 * W], dt.bfloat16, tag="pt")
                    for r in range(2):
                        k = 2 * g + r
                        nc.tensor.matmul(
                            ps[:, r * W:(r + 1) * W],
                            kt_chunk(k),
                            qslc, start=True, stop=True)
                    nc.scalar.activation(ptg[:], ps[:], AF.Exp,
                                         scale=SCALE, bias=negC[:])
                    if g >= nkc // 2 - 2:
                        gg = g - (nkc // 2 - 2)   # 0 or 1
                        nc.vector.tensor_mul(
                            ptg[:], ptg[:],
                            msk_sb[:, gg * 2 * W:(gg + 1) * 2 * W])
                    if g == 0:
                        nc.vector.tensor_copy(dacc[:], ptg[:])
                    else:
                        nc.vector.tensor_add(dacc[:], dacc[:], ptg[:])
                    for r in range(2):
                        k = 2 * g + r
                        nc.tensor.matmul(
                            ot[:], vn_chunk(k),
                            ptg[:, r * W:(r + 1) * W],
                            start=(k == 0), stop=(k == nkc - 1))
                    if spreader is not None:
                        spreader.tick()
                dn = ppv.tile([128, W], dt.float32, tag="ppv_ps")
                nc.tensor.matmul(dn[0:1, :], ones_sb[:, 0:1],
                                 dacc[:, 0:W], start=True, stop=False)
                nc.tensor.matmul(dn[0:1, :], ones_sb[:, 0:1],
                                 dacc[:, W:2 * W], start=False, stop=True)
                drc = workp.tile([1, W], dt.float32, tag="drc")
                nc.vector.reciprocal_approx_fast(drc[:], dn[0:1, :])
                drb = workp.tile([128, W], dt.float32, tag="drb")
                nc.gpsimd.partition_broadcast(drb[:], drc[:])
                ots = otsp.tile([128, W], dt.bfloat16, tag="ots")
                nc.vector.tensor_mul(ots[:], ot[:], drb[:])
                return ots

            def oproj_qc(j, ots_heads, qc):
                ob = obp.tile([128, H], dt.float32, tag="ob")
                for n in range(H // W):
                    po = pp.tile([128, W], dt.float32, tag="pp_ps")
                    for a in range(2):
                        nc.tensor.matmul(
                            po[:],
                            ots_heads[a][:, qc * 128:(qc + 1) * 128],
                            wo_sb[:, a * H + n * W: a * H + (n + 1) * W],
                            start=(a == 0), stop=(a == 1))
                    nc.vector.tensor_copy(ob[:, n * W:(n + 1) * W], po[:])
                nc.sync.dma_start(
                    out[j * W + qc * 128: j * W + (qc + 1) * 128, :], ob[:])

            # o_proj(j-1) matmuls have no exp dependency, so they are spread
            # across window j's group loops as PE filler for the exp-paced
            # stretches (the exp of group g gates PV(g); oproj work keeps the
            # PE busy in the meantime)
            class OprojSpreader:
                def __init__(self, j, ots_heads, total_groups):
                    self.j, self.ots = j, ots_heads
                    self.total = max(total_groups, 4)
                    self.seen = 0
                    self.done = 0

                def tick(self):
                    self.seen += 1
                    target = min(4, self.seen * 4 // self.total)
                    while self.done < target:
                        oproj_qc(self.j, self.ots, self.done)
                        self.done += 1

                def finish(self):
                    while self.done < 4:
                        oproj_qc(self.j, self.ots, self.done)
                        self.done += 1

            prev = None
            for j in range(NW):
                spreader = None
                if prev is not None:
                    spreader = OprojSpreader(j - 1, prev, 4 * j + 4)
                o0 = attn_head(0, j, spreader)
                o1 = attn_head(1, j, spreader)
                if spreader is not None:
                    spreader.finish()
                prev = (o0, o1)
            spreader = OprojSpreader(NW - 1, prev, 4)
            spreader.finish()

    nc.compile()
    return nc


def _prep_inputs(x, cos, sin, Wq, bq, Wk, bk, Wv, bv, Wo):
    x = np.asarray(x, dtype=np.float32).reshape(S, H)
    cos = np.asarray(cos, dtype=np.float32).reshape(S, DH)
    sin = np.asarray(sin, dtype=np.float32).reshape(S, DH)

    xtT = x.T.astype(BF16)                       # [H, S]
    # blocked layout: [seq_block, partition, hid_chunk * W] so each block's
    # DMA is one fully-contiguous read
    xtb = np.ascontiguousarray(
        xtT.reshape(HC, 128, NW, W).transpose(2, 1, 0, 3).reshape(NW, 128, HC * W))

    # head-dim permutation: partition 2t <- dim t, partition 2t+1 <- dim t+64
    perm = np.empty(DH, np.int64)
    perm[0::2] = np.arange(64)
    perm[1::2] = np.arange(64) + 64

    cosT = np.ascontiguousarray(cos.T)          # [128, S]
    sinT = np.ascontiguousarray(sin.T)
    cosP = np.ascontiguousarray(cosT[perm]).astype(BF16)
    sinsP = np.empty_like(sinT)
    sinsP[0::2] = -sinT[:64]
    sinsP[1::2] = sinT[:64]
    sinsP = np.ascontiguousarray(sinsP).astype(BF16)

    # causal 0/1 masks for the 4 diagonal-band chunks of each 512-q window
    kk = np.arange(128)[:, None]
    qq = np.arange(W)[None, :]
    msk = np.concatenate(
        [(qq >= kk + 128 * r).astype(np.float32) for r in range(4)],
        axis=1).astype(BF16)

    Wq = np.asarray(Wq, np.float32)
    Wk = np.asarray(Wk, np.float32)
    Wv = np.asarray(Wv, np.float32)
    Wo = np.asarray(Wo, np.float32)
    bq = np.asarray(bq, np.float32)
    bk = np.asarray(bk, np.float32)
    bv = np.asarray(bv, np.float32)

    in_maps = []
    for c in range(N_CORES):
        kv = c // 4
        qtr = c % 4                      # this core's K/V sequence quarter
        # q/k projections get the RoPE head-dim permutation applied to their
        # output columns (and biases); v/o stay in natural order
        wq_c = np.concatenate(
            [Wq[:, (2 * c + a) * DH:(2 * c + a + 1) * DH][:, perm]
             for a in range(2)], axis=1)
        wk_c = Wk[:, kv * DH:(kv + 1) * DH][:, perm]
        wv_c = Wv[:, kv * DH:(kv + 1) * DH]
        wo_c = Wo[2 * c * DH:(2 * c + 2) * DH, :]
        bq_c = np.stack(
            [bq[(2 * c + a) * DH:(2 * c + a + 1) * DH][perm] for a in range(2)],
            axis=1)
        bkv_c = np.stack(
            [bk[kv * DH:(kv + 1) * DH][perm], bv[kv * DH:(kv + 1) * DH]],
            axis=1)
        def wrearr(w):
            c = w.shape[0] // 128
            return np.ascontiguousarray(
                w.reshape(c, 128, -1).transpose(1, 0, 2).reshape(128, -1))

        in_maps.append({
            "xt": xtb,
            "wq": wrearr(wq_c).astype(BF16),
            "wk": wrearr(wk_c).astype(BF16),
            "wv": wrearr(wv_c).astype(BF16),
            "wo": wrearr(wo_c).astype(BF16),
            "bq": np.ascontiguousarray(bq_c).astype(np.float32),
            "bkv": np.ascontiguousarray(bkv_c).astype(np.float32),
            "cost": cosP, "sins": sinsP,
            "msk": msk,
        })
    return in_maps


def _get_nc():
    if "nc" not in _CACHE:
        _CACHE["nc"] = _build()
    return _CACHE["nc"]


def run(trace=False, tmpdir=None, **inputs):
    from concourse.bass_utils import run_bass_kernel_spmd

    nc = _get_nc()
    in_maps = _prep_inputs(**inputs)
    kw = {}
    if trace:
        kw = dict(trace=True, tmpdir=tmpdir)
    res = run_bass_kernel_spmd(nc, in_maps, core_ids=list(range(N_CORES)), **kw)
    acc = np.zeros((S, H), dtype=np.float32)
    for r in res.results:
        acc += r["out"]
    return acc.reshape(1, S, H), res


def kernel(**inputs) -> np.ndarray:
    out, _ = run(**inputs)
    return out


# revision 33
# speedup vs baseline: 1.1454x; 1.1454x over previous
"""GQA attention block (16 q heads / 2 kv heads, RoPE, causal) on 8 TRN2 NeuronCores.

Strategy: tensor-parallel over heads. Each core owns 2 q heads + the matching
kv head (kv heads replicated over 4-core groups), computes its partial o_proj
output over the full sequence, and the host sums the 8 partials. All cores run
the identical graph; only the input *data* differs per core (SPMD-safe).

Dataflow (everything "transposed" so no on-chip transpose of activations is
ever needed):
  - host passes x^T (bf16) pre-blocked per 512-seq window so every DMA is one
    contiguous read; weights are host-rearranged to [128, chunk*cols] likewise
  - projections compute Q^T (2 heads), K^T and V per window with the weight
    chunk stationary and x^T streaming; K/V are computed by every core
    (replicating them via a 4-core AllGather measured slower than recompute)
  - RoPE head-dim is host-permuted so rotate-half partners sit on adjacent
    partitions: the swap is a DVE within-quadrant stream_shuffle; bias adds
    ride a DVE tensor_scalar_add out of PSUM
  - scores are computed transposed: S^T[key, q] = K^T_chunk.T @ Q^T
  - softmax without max-subtraction, shifted: P = exp(s*scale - 6) on ACT,
    written straight to bf16 (|scaled scores| < 6 so this is exact enough and
    the shift cancels in the ratio)
  - causal masking multiplies the diagonal-band chunks with 0/1 masks (DVE)
  - denominator: fp16 accumulation of P^T groups on DVE (fp16 keeps the 2x
    perf mode; bf16 would too but fp16 halves the rounding error), a
    ones-vector matmul partition-reduce, reciprocal_approx_fast, and a
    gpsimd partition_broadcast; the scale folds into the out^T -> SBUF copy
  - PV accumulates out^T[d, q] with V (natural layout, via PE transpose)
    stationary and P^T streaming
  - o_proj uses out^T slices as the stationary operand directly; o_proj for
    window j-1 is interleaved between the two heads of window j to keep PE fed

Measured: 386 us HW exec (neuron-profile), rel l2 err 5.3e-3 vs the fp32
reference. PE is the bottleneck (median matmul spacing at the 216 ns N=512
hardware floor; ~33 us total PE idle).
"""

import os
import sys

for _p in ("/opt/trn_rl_repo",):
    if os.path.isdir(_p) and _p not in sys.path:
        sys.path.append(_p)

import numpy as np
import ml_dtypes

FP16 = np.float16
BF16 = ml_dtypes.bfloat16

# ---- problem constants (hardcoded per harness contract) ----
S = 4096          # sequence length
H = 2048          # hidden
DH = 128          # head dim
N_CORES = 8
HC = H // 128     # 16 hidden chunks
W = 512           # q-window width
NW = S // W       # 8 windows
SQ = S // 4       # sequence quarter (per-core K/V share)
SCALE = 1.0 / float(np.sqrt(DH))
EXP_SHIFT = -6.0

_CACHE = {}


def _build():
    import concourse.bacc as bacc
    import concourse.mybir as mybir
    import concourse.tile as tile
    from concourse.masks import make_identity

    dt = mybir.dt
    AF = mybir.ActivationFunctionType

    nc = bacc.Bacc("TRN2", target_bir_lowering=False, debug=False,
                   num_devices=N_CORES)

    xt = nc.dram_tensor("xt", [NW, 128, HC * W], dt.bfloat16, kind="ExternalInput")
    wq = nc.dram_tensor("wq", [128, HC * 2 * DH], dt.bfloat16, kind="ExternalInput")
    wk = nc.dram_tensor("wk", [128, HC * DH], dt.bfloat16, kind="ExternalInput")
    wv = nc.dram_tensor("wv", [128, HC * DH], dt.bfloat16, kind="ExternalInput")
    wo = nc.dram_tensor("wo", [128, 2 * H], dt.bfloat16, kind="ExternalInput")
    bqd = nc.dram_tensor("bq", [128, 2], dt.float32, kind="ExternalInput")
    bkvd = nc.dram_tensor("bkv", [128, 2], dt.float32, kind="ExternalInput")
    cosd = nc.dram_tensor("cost", [128, S], dt.bfloat16, kind="ExternalInput")
    sind = nc.dram_tensor("sins", [128, S], dt.bfloat16, kind="ExternalInput")
    mskd = nc.dram_tensor("msk", [128, 4 * W], dt.bfloat16, kind="ExternalInput")
    out = nc.dram_tensor("out", [S, H], dt.float32, kind="ExternalOutput")

    with tile.TileContext(nc) as tc:
        with (
            tc.tile_pool(name="const", bufs=1) as constp,
            tc.tile_pool(name="xtp", bufs=2) as xtp,
            tc.tile_pool(name="proj", bufs=1) as projp,
            tc.tile_pool(name="ptp", bufs=8) as ptp,
            tc.tile_pool(name="work", bufs=2) as workp,
            tc.tile_pool(name="otsp", bufs=5) as otsp,
            tc.tile_pool(name="obp", bufs=2) as obp,
            tc.tile_pool(name="pp", bufs=2, space="PSUM") as pp,
            tc.tile_pool(name="pqk", bufs=2, space="PSUM") as pqk,
            tc.tile_pool(name="ppv", bufs=2, space="PSUM") as ppv,
        ):
            # ---------- constants into SBUF ----------
            wq_sb = constp.tile([128, HC * 2 * DH], dt.bfloat16, tag="wq")
            wk_sb = constp.tile([128, HC * DH], dt.bfloat16, tag="wk")
            wv_sb = constp.tile([128, HC * DH], dt.bfloat16, tag="wv")
            wo_sb = constp.tile([128, 2 * H], dt.bfloat16, tag="wo")
            bq_sb = constp.tile([128, 2], dt.float32, tag="bq")
            bkv_sb = constp.tile([128, 2], dt.float32, tag="bkv")
            cos_sb = constp.tile([128, S], dt.bfloat16, tag="cos")
            sin_sb = constp.tile([128, S], dt.bfloat16, tag="sin")
            msk_sb = constp.tile([128, 4 * W], dt.bfloat16, tag="msk")
            ones_sb = constp.tile([128, 1], dt.float16, tag="ones")
            ident = constp.tile([128, 128], dt.bfloat16, tag="ident")
            negC = constp.tile([128, 1], dt.float32, tag="negC")

            nc.sync.dma_start(wq_sb[:], wq[:, :])
            nc.gpsimd.dma_start(wk_sb[:], wk[:, :])
            nc.gpsimd.dma_start(wv_sb[:], wv[:, :])
            nc.gpsimd.dma_start(bq_sb[:], bqd[:, :])
            nc.gpsimd.dma_start(bkv_sb[:], bkvd[:, :])
            nc.gpsimd.dma_start(cos_sb[:], cosd[:, :])
            nc.gpsimd.dma_start(sin_sb[:], sind[:, :])
            nc.gpsimd.dma_start(msk_sb[:], mskd[:, :])
            nc.gpsimd.dma_start(wo_sb[:], wo[:, :])
            nc.gpsimd.memset(ones_sb[:], 1.0)
            nc.gpsimd.memset(negC[:], EXP_SHIFT)
            make_identity(nc, ident[:])

            qt_sb = projp.tile([128, 2 * S], dt.bfloat16, tag="qt")
            kt_q = [projp.tile([128, SQ], dt.bfloat16, tag=f"ktq{r}",
                               name=f"ktq{r}") for r in range(4)]
            vn_q = [projp.tile([128, SQ], dt.bfloat16, tag=f"vnq{r}",
                               name=f"vnq{r}") for r in range(4)]

            def kt_chunk(k):
                return kt_q[k // 8][:, (k % 8) * 128:(k % 8 + 1) * 128]

            def vn_chunk(k):
                return vn_q[k // 8][:, (k % 8) * 128:(k % 8 + 1) * 128]

            shuffle_mask = [i ^ 1 for i in range(32)]

            def rope_store(ps, bias, dest_slc, cslc, sslc):
                t0 = workp.tile([128, W], dt.bfloat16, tag="rope0")
                nc.vector.tensor_scalar_add(t0[:], ps[:], bias)
                tsw = workp.tile([128, W], dt.bfloat16, tag="ropesw")
                nc.vector.stream_shuffle(tsw[:], t0[:], mask=shuffle_mask)
                t1 = workp.tile([128, W], dt.bfloat16, tag="rope1")
                nc.vector.tensor_mul(t1[:], t0[:], cslc)
                t2 = workp.tile([128, W], dt.bfloat16, tag="rope2")
                nc.vector.tensor_mul(t2[:], tsw[:], sslc)
                nc.vector.tensor_add(dest_slc, t1[:], t2[:])

            # ---------- phase 1: Q^T (2 heads) + K^T + V projections ----------
            for sb in range(NW):
                xb = xtp.tile([128, HC * W], dt.bfloat16, tag="xtb")
                nc.sync.dma_start(xb[:], xt[sb, :, :])
                targets = [
                    ("rope", lambda h: wq_sb[:, h * 256:h * 256 + 128],
                     bq_sb[:, 0:1], qt_sb, 0, cos_sb, sin_sb),
                    ("rope", lambda h: wq_sb[:, h * 256 + 128:h * 256 + 256],
                     bq_sb[:, 1:2], qt_sb, S, cos_sb, sin_sb),
                    ("rope", lambda h: wk_sb[:, h * 128:(h + 1) * 128],
                     bkv_sb[:, 0:1], kt_q[sb // 2], -(sb // 2) * 2 * W,
                     cos_sb, sin_sb),
                    ("vnat", lambda h: wv_sb[:, h * 128:(h + 1) * 128],
                     bkv_sb[:, 1:2], vn_q[sb // 2], 0, None, None),
                ]
                for kind, wslc, bias, dest, doff, ctab, stab in targets:
                    ps = pp.tile([128, W], dt.float32, tag="pp_ps")
                    for h in range(HC):
                        nc.tensor.matmul(
                            ps[:], wslc(h), xb[:, h * W:(h + 1) * W],
                            start=(h == 0), stop=(h == HC - 1))
                    if kind == "rope":
                        rope_store(ps, bias,
                                   dest[:, doff + sb * W: doff + (sb + 1) * W],
                                   ctab[:, sb * W:(sb + 1) * W],
                                   stab[:, sb * W:(sb + 1) * W])
                    else:
                        t0 = workp.tile([128, W], dt.bfloat16, tag="vstage")
                        nc.vector.tensor_scalar_add(t0[:], ps[:], bias)
                        for i in range(W // 128):
                            tp = ppv.tile([128, 128], dt.bfloat16, tag="ppv_ps")
                            nc.tensor.transpose(
                                tp[:], t0[:, i * 128:(i + 1) * 128], ident[:])
                            nc.vector.tensor_copy(
                                dest[:, ((sb % 2) * 4 + i) * 128:
                                     ((sb % 2) * 4 + i + 1) * 128],
                                tp[:])

            # ---------- phase 2: attention, with o_proj(j-1) interleaved ----------
            def attn_head(a, j):
                nkc = 4 * j + 4
                qslc = qt_sb[:, a * S + j * W: a * S + (j + 1) * W]
                ot = ppv.tile([128, W], dt.float32, tag="ppv_ps")
                dacc = workp.tile([128, 2 * W], dt.float16, tag="dacc")
                for g in range(nkc // 2):
                    ps = pqk.tile([128, 2 * W], dt.float32, tag="qk_ps")
                    ptg = ptp.tile([128, 2 * W], dt.bfloat16, tag="pt")
                    for r in range(2):
                        k = 2 * g + r
                        nc.tensor.matmul(
                            ps[:, r * W:(r + 1) * W],
                            kt_chunk(k),
                            qslc, start=True, stop=True)
                    nc.scalar.activation(ptg[:], ps[:], AF.Exp,
                                         scale=SCALE, bias=negC[:])
                    if g >= nkc // 2 - 2:
                        gg = g - (nkc // 2 - 2)   # 0 or 1
                        nc.vector.tensor_mul(
                            ptg[:], ptg[:],
                            msk_sb[:, gg * 2 * W:(gg + 1) * 2 * W])
                    if g == 0:
                        nc.vector.tensor_copy(dacc[:], ptg[:])
                    else:
                        nc.vector.tensor_add(dacc[:], dacc[:], ptg[:])
                    for r in range(2):
                        k = 2 * g + r
                        nc.tensor.matmul(
                            ot[:], vn_chunk(k),
                            ptg[:, r * W:(r + 1) * W],
                            start=(k == 0), stop=(k == nkc - 1))
                dn = ppv.tile([128, W], dt.float32, tag="ppv_ps")
                nc.tensor.matmul(dn[0:1, :], ones_sb[:, 0:1],
                                 dacc[:, 0:W], start=True, stop=False)
                nc.tensor.matmul(dn[0:1, :], ones_sb[:, 0:1],
                                 dacc[:, W:2 * W], start=False, stop=True)
                drc = workp.tile([1, W], dt.float32, tag="drc")
                nc.vector.reciprocal_approx_fast(drc[:], dn[0:1, :])
                drb = workp.tile([128, W], dt.float32, tag="drb")
                nc.gpsimd.partition_broadcast(drb[:], drc[:])
                ots = otsp.tile([128, W], dt.bfloat16, tag="ots")
                nc.vector.tensor_mul(ots[:], ot[:], drb[:])
                return ots

            def oproj_qc(j, ots_heads, qc):
                ob = obp.tile([128, H], dt.float32, tag="ob")
                for n in range(H // W):
                    po = pp.tile([128, W], dt.float32, tag="pp_ps")
                    for a in range(2):
                        nc.tensor.matmul(
                            po[:],
                            ots_heads[a][:, qc * 128:(qc + 1) * 128],
                            wo_sb[:, a * H + n * W: a * H + (n + 1) * W],
                            start=(a == 0), stop=(a == 1))
                    nc.vector.tensor_copy(ob[:, n * W:(n + 1) * W], po[:])
                nc.sync.dma_start(
                    out[j * W + qc * 128: j * W + (qc + 1) * 128, :], ob[:])

            # o_proj(j-1) runs as one contiguous block between the two
            # heads of window j: the PE's LDWEIGHTS pipelining only sustains
            # the 216 ns matmul pace for contiguous runs of one weight
            # stream, so fine-grained interleaving is a net loss (measured)
            prev = None
            for j in range(NW):
                o0 = attn_head(0, j)
                if prev is not None:
                    for qc in range(W // 128):
                        oproj_qc(j - 1, prev, qc)
                o1 = attn_head(1, j)
                prev = (o0, o1)
            for qc in range(W // 128):
                oproj_qc(NW - 1, prev, qc)

    nc.compile()
    return nc


def _prep_inputs(x, cos, sin, Wq, bq, Wk, bk, Wv, bv, Wo):
    x = np.asarray(x, dtype=np.float32).reshape(S, H)
    cos = np.asarray(cos, dtype=np.float32).reshape(S, DH)
    sin = np.asarray(sin, dtype=np.float32).reshape(S, DH)

    xtT = x.T.astype(BF16)                       # [H, S]
    # blocked layout: [seq_block, partition, hid_chunk * W] so each block's
    # DMA is one fully-contiguous read
    xtb = np.ascontiguousarray(
        xtT.reshape(HC, 128, NW, W).transpose(2, 1, 0, 3).reshape(NW, 128, HC * W))

    # head-dim permutation: partition 2t <- dim t, partition 2t+1 <- dim t+64
    perm = np.empty(DH, np.int64)
    perm[0::2] = np.arange(64)
    perm[1::2] = np.arange(64) + 64

    cosT = np.ascontiguousarray(cos.T)          # [128, S]
    sinT = np.ascontiguousarray(sin.T)
    cosP = np.ascontiguousarray(cosT[perm]).astype(BF16)
    sinsP = np.empty_like(sinT)
    sinsP[0::2] = -sinT[:64]
    sinsP[1::2] = sinT[:64]
    sinsP = np.ascontiguousarray(sinsP).astype(BF16)

    # causal 0/1 masks for the 4 diagonal-band chunks of each 512-q window
    kk = np.arange(128)[:, None]
    qq = np.arange(W)[None, :]
    msk = np.concatenate(
        [(qq >= kk + 128 * r).astype(np.float32) for r in range(4)],
        axis=1).astype(BF16)

    Wq = np.asarray(Wq, np.float32)
    Wk = np.asarray(Wk, np.float32)
    Wv = np.asarray(Wv, np.float32)
    Wo = np.asarray(Wo, np.float32)
    bq = np.asarray(bq, np.float32)
    bk = np.asarray(bk, np.float32)
    bv = np.asarray(bv, np.float32)

    in_maps = []
    for c in range(N_CORES):
        kv = c // 4
        qtr = c % 4                      # this core's K/V sequence quarter
        # q/k projections get the RoPE head-dim permutation applied to their
        # output columns (and biases); v/o stay in natural order
        wq_c = np.concatenate(
            [Wq[:, (2 * c + a) * DH:(2 * c + a + 1) * DH][:, perm]
             for a in range(2)], axis=1)
        wk_c = Wk[:, kv * DH:(kv + 1) * DH][:, perm]
        wv_c = Wv[:, kv * DH:(kv + 1) * DH]
        wo_c = Wo[2 * c * DH:(2 * c + 2) * DH, :]
        bq_c = np.stack(
            [bq[(2 * c + a) * DH:(2 * c + a + 1) * DH][perm] for a in range(2)],
            axis=1)
        bkv_c = np.stack(
            [bk[kv * DH:(kv + 1) * DH][perm], bv[kv * DH:(kv + 1) * DH]],
            axis=1)
        def wrearr(w):
            c = w.shape[0] // 128
            return np.ascontiguousarray(
                w.reshape(c, 128, -1).transpose(1, 0, 2).reshape(128, -1))

        in_maps.append({
            "xt": xtb,
            "wq": wrearr(wq_c).astype(BF16),
            "wk": wrearr(wk_c).astype(BF16),
            "wv": wrearr(wv_c).astype(BF16),
            "wo": wrearr(wo_c).astype(BF16),
            "bq": np.ascontiguousarray(bq_c).astype(np.float32),
            "bkv": np.ascontiguousarray(bkv_c).astype(np.float32),
            "cost": cosP, "sins": sinsP,
            "msk": msk,
        })
    return in_maps


def _get_nc():
    if "nc" not in _CACHE:
        _CACHE["nc"] = _build()
    return _CACHE["nc"]


def run(trace=False, tmpdir=None, **inputs):
    from concourse.bass_utils import run_bass_kernel_spmd

    nc = _get_nc()
    in_maps = _prep_inputs(**inputs)
    kw = {}
    if trace:
        kw = dict(trace=True, tmpdir=tmpdir)
    res = run_bass_kernel_spmd(nc, in_maps, core_ids=list(range(N_CORES)), **kw)
    acc = np.zeros((S, H), dtype=np.float32)
    for r in res.results:
        acc += r["out"]
    return acc.reshape(1, S, H), res


def kernel(**inputs) -> np.ndarray:
    out, _ = run(**inputs)
    return out


# revision 34
# speedup vs baseline: 1.1517x; 1.0055x over previous
"""GQA attention block (16 q heads / 2 kv heads, RoPE, causal) on 8 TRN2 NeuronCores.

Strategy: tensor-parallel over heads. Each core owns 2 q heads + the matching
kv head (kv heads replicated over 4-core groups), computes its partial o_proj
output over the full sequence, and the host sums the 8 partials. All cores run
the identical graph; only the input *data* differs per core (SPMD-safe).

Dataflow (everything "transposed" so no on-chip transpose of activations is
ever needed):
  - host passes x^T (bf16) pre-blocked per 512-seq window so every DMA is one
    contiguous read; weights are host-rearranged to [128, chunk*cols] likewise
  - projections compute Q^T (2 heads), K^T and V per window with the weight
    chunk stationary and x^T streaming; K/V are computed by every core
    (replicating them via a 4-core AllGather measured slower than recompute)
  - RoPE head-dim is host-permuted so rotate-half partners sit on adjacent
    partitions: the swap is a DVE within-quadrant stream_shuffle; bias adds
    ride a DVE tensor_scalar_add out of PSUM
  - scores are computed transposed: S^T[key, q] = K^T_chunk.T @ Q^T
  - softmax without max-subtraction, shifted: P = exp(s*scale - 6) on ACT,
    written straight to bf16 (|scaled scores| < 6 so this is exact enough and
    the shift cancels in the ratio)
  - causal masking multiplies the diagonal-band chunks with 0/1 masks (DVE)
  - denominator: fp16 accumulation of P^T groups on DVE (fp16 keeps the 2x
    perf mode; bf16 would too but fp16 halves the rounding error), a
    ones-vector matmul partition-reduce, reciprocal_approx_fast, and a
    gpsimd partition_broadcast; the scale folds into the out^T -> SBUF copy
  - PV accumulates out^T[d, q] with V (natural layout, via PE transpose)
    stationary and P^T streaming
  - o_proj uses out^T slices as the stationary operand directly; o_proj for
    window j-1 is interleaved between the two heads of window j to keep PE fed

Measured: 386 us HW exec (neuron-profile), rel l2 err 5.3e-3 vs the fp32
reference. PE is the bottleneck (median matmul spacing at the 216 ns N=512
hardware floor; ~33 us total PE idle).
"""

import os
import sys

for _p in ("/opt/trn_rl_repo",):
    if os.path.isdir(_p) and _p not in sys.path:
        sys.path.append(_p)

import numpy as np
import ml_dtypes

FP16 = np.float16
BF16 = ml_dtypes.bfloat16

# ---- problem constants (hardcoded per harness contract) ----
S = 4096          # sequence length
H = 2048          # hidden
DH = 128          # head dim
N_CORES = 8
HC = H // 128     # 16 hidden chunks
W = 512           # q-window width
NW = S // W       # 8 windows
SQ = S // 4       # sequence quarter (per-core K/V share)
SCALE = 1.0 / float(np.sqrt(DH))
EXP_SHIFT = -6.0

_CACHE = {}


def _build():
    import concourse.bacc as bacc
    import concourse.mybir as mybir
    import concourse.tile as tile
    from concourse.masks import make_identity

    dt = mybir.dt
    AF = mybir.ActivationFunctionType

    nc = bacc.Bacc("TRN2", target_bir_lowering=False, debug=False,
                   num_devices=N_CORES)

    xt = nc.dram_tensor("xt", [NW, 128, HC * W], dt.bfloat16, kind="ExternalInput")
    wq = nc.dram_tensor("wq", [128, HC * 2 * DH], dt.bfloat16, kind="ExternalInput")
    wk = nc.dram_tensor("wk", [128, HC * DH], dt.bfloat16, kind="ExternalInput")
    wv = nc.dram_tensor("wv", [128, HC * DH], dt.bfloat16, kind="ExternalInput")
    wo = nc.dram_tensor("wo", [128, 2 * H], dt.bfloat16, kind="ExternalInput")
    bqd = nc.dram_tensor("bq", [128, 2], dt.float32, kind="ExternalInput")
    bkvd = nc.dram_tensor("bkv", [128, 2], dt.float32, kind="ExternalInput")
    cosd = nc.dram_tensor("cost", [128, S], dt.bfloat16, kind="ExternalInput")
    sind = nc.dram_tensor("sins", [128, S], dt.bfloat16, kind="ExternalInput")
    mskd = nc.dram_tensor("msk", [128, 4 * W], dt.bfloat16, kind="ExternalInput")
    out = nc.dram_tensor("out", [S, H], dt.float32, kind="ExternalOutput")

    with tile.TileContext(nc) as tc:
        with (
            tc.tile_pool(name="const", bufs=1) as constp,
            tc.tile_pool(name="xtp", bufs=2) as xtp,
            tc.tile_pool(name="proj", bufs=1) as projp,
            tc.tile_pool(name="ptp", bufs=8) as ptp,
            tc.tile_pool(name="work", bufs=2) as workp,
            tc.tile_pool(name="otsp", bufs=5) as otsp,
            tc.tile_pool(name="obp", bufs=2) as obp,
            tc.tile_pool(name="pp", bufs=2, space="PSUM") as pp,
            tc.tile_pool(name="pqk", bufs=2, space="PSUM") as pqk,
            tc.tile_pool(name="ppv", bufs=2, space="PSUM") as ppv,
        ):
            # ---------- constants into SBUF ----------
            wq_sb = constp.tile([128, HC * 2 * DH], dt.bfloat16, tag="wq")
            wk_sb = constp.tile([128, HC * DH], dt.bfloat16, tag="wk")
            wv_sb = constp.tile([128, HC * DH], dt.bfloat16, tag="wv")
            wo_sb = constp.tile([128, 2 * H], dt.bfloat16, tag="wo")
            bq_sb = constp.tile([128, 2], dt.float32, tag="bq")
            bkv_sb = constp.tile([128, 2], dt.float32, tag="bkv")
            cos_sb = constp.tile([128, S], dt.bfloat16, tag="cos")
            sin_sb = constp.tile([128, S], dt.bfloat16, tag="sin")
            msk_sb = constp.tile([128, 4 * W], dt.bfloat16, tag="msk")
            ones_sb = constp.tile([128, 1], dt.float16, tag="ones")
            ident = constp.tile([128, 128], dt.bfloat16, tag="ident")
            negC = constp.tile([128, 1], dt.float32, tag="negC")

            # first slice covers hid chunks 0-3 so the first projection
            # matmuls only wait on 0.25 MB; the rest lands right behind xb0
            nc.sync.dma_start(wq_sb[:, 0:1024], wq[:, 0:1024])
            nc.gpsimd.dma_start(wk_sb[:], wk[:, :])
            nc.gpsimd.dma_start(wv_sb[:], wv[:, :])
            nc.gpsimd.dma_start(bq_sb[:], bqd[:, :])
            nc.gpsimd.dma_start(bkv_sb[:], bkvd[:, :])
            nc.gpsimd.dma_start(cos_sb[:], cosd[:, :])
            nc.gpsimd.dma_start(sin_sb[:], sind[:, :])
            nc.gpsimd.dma_start(msk_sb[:], mskd[:, :])
            nc.gpsimd.dma_start(wo_sb[:], wo[:, :])
            nc.gpsimd.memset(ones_sb[:], 1.0)
            nc.gpsimd.memset(negC[:], EXP_SHIFT)
            make_identity(nc, ident[:])

            qt_sb = projp.tile([128, 2 * S], dt.bfloat16, tag="qt")
            kt_q = [projp.tile([128, SQ], dt.bfloat16, tag=f"ktq{r}",
                               name=f"ktq{r}") for r in range(4)]
            vn_q = [projp.tile([128, SQ], dt.bfloat16, tag=f"vnq{r}",
                               name=f"vnq{r}") for r in range(4)]

            def kt_chunk(k):
                return kt_q[k // 8][:, (k % 8) * 128:(k % 8 + 1) * 128]

            def vn_chunk(k):
                return vn_q[k // 8][:, (k % 8) * 128:(k % 8 + 1) * 128]

            shuffle_mask = [i ^ 1 for i in range(32)]

            def rope_store(ps, bias, dest_slc, cslc, sslc):
                t0 = workp.tile([128, W], dt.bfloat16, tag="rope0")
                nc.vector.tensor_scalar_add(t0[:], ps[:], bias)
                tsw = workp.tile([128, W], dt.bfloat16, tag="ropesw")
                nc.vector.stream_shuffle(tsw[:], t0[:], mask=shuffle_mask)
                t1 = workp.tile([128, W], dt.bfloat16, tag="rope1")
                nc.vector.tensor_mul(t1[:], t0[:], cslc)
                t2 = workp.tile([128, W], dt.bfloat16, tag="rope2")
                nc.vector.tensor_mul(t2[:], tsw[:], sslc)
                nc.vector.tensor_add(dest_slc, t1[:], t2[:])

            # ---------- phase 1: Q^T (2 heads) + K^T + V projections ----------
            for sb in range(NW):
                xb = xtp.tile([128, HC * W], dt.bfloat16, tag="xtb")
                nc.sync.dma_start(xb[:], xt[sb, :, :])
                if sb == 0:
                    nc.sync.dma_start(wq_sb[:, 1024:], wq[:, 1024:])
                targets = [
                    ("rope", lambda h: wq_sb[:, h * 256:h * 256 + 128],
                     bq_sb[:, 0:1], qt_sb, 0, cos_sb, sin_sb),
                    ("rope", lambda h: wq_sb[:, h * 256 + 128:h * 256 + 256],
                     bq_sb[:, 1:2], qt_sb, S, cos_sb, sin_sb),
                    ("rope", lambda h: wk_sb[:, h * 128:(h + 1) * 128],
                     bkv_sb[:, 0:1], kt_q[sb // 2], -(sb // 2) * 2 * W,
                     cos_sb, sin_sb),
                    ("vnat", lambda h: wv_sb[:, h * 128:(h + 1) * 128],
                     bkv_sb[:, 1:2], vn_q[sb // 2], 0, None, None),
                ]
                for kind, wslc, bias, dest, doff, ctab, stab in targets:
                    ps = pp.tile([128, W], dt.float32, tag="pp_ps")
                    for h in range(HC):
                        nc.tensor.matmul(
                            ps[:], wslc(h), xb[:, h * W:(h + 1) * W],
                            start=(h == 0), stop=(h == HC - 1))
                    if kind == "rope":
                        rope_store(ps, bias,
                                   dest[:, doff + sb * W: doff + (sb + 1) * W],
                                   ctab[:, sb * W:(sb + 1) * W],
                                   stab[:, sb * W:(sb + 1) * W])
                    else:
                        t0 = workp.tile([128, W], dt.bfloat16, tag="vstage")
                        nc.vector.tensor_scalar_add(t0[:], ps[:], bias)
                        for i in range(W // 128):
                            tp = ppv.tile([128, 128], dt.bfloat16, tag="ppv_ps")
                            nc.tensor.transpose(
                                tp[:], t0[:, i * 128:(i + 1) * 128], ident[:])
                            nc.vector.tensor_copy(
                                dest[:, ((sb % 2) * 4 + i) * 128:
                                     ((sb % 2) * 4 + i + 1) * 128],
                                tp[:])

            # ---------- phase 2: attention, with o_proj(j-1) interleaved ----------
            def attn_head(a, j):
                nkc = 4 * j + 4
                qslc = qt_sb[:, a * S + j * W: a * S + (j + 1) * W]
                ot = ppv.tile([128, W], dt.float32, tag="ppv_ps")
                dacc = workp.tile([128, 2 * W], dt.float16, tag="dacc")
                for g in range(nkc // 2):
                    ps = pqk.tile([128, 2 * W], dt.float32, tag="qk_ps")
                    ptg = ptp.tile([128, 2 * W], dt.bfloat16, tag="pt")
                    for r in range(2):
                        k = 2 * g + r
                        nc.tensor.matmul(
                            ps[:, r * W:(r + 1) * W],
                            kt_chunk(k),
                            qslc, start=True, stop=True)
                    nc.scalar.activation(ptg[:], ps[:], AF.Exp,
                                         scale=SCALE, bias=negC[:])
                    if g >= nkc // 2 - 2:
                        gg = g - (nkc // 2 - 2)   # 0 or 1
                        nc.vector.tensor_mul(
                            ptg[:], ptg[:],
                            msk_sb[:, gg * 2 * W:(gg + 1) * 2 * W])
                    if g == 0:
                        nc.vector.tensor_copy(dacc[:], ptg[:])
                    else:
                        nc.vector.tensor_add(dacc[:], dacc[:], ptg[:])
                    for r in range(2):
                        k = 2 * g + r
                        nc.tensor.matmul(
                            ot[:], vn_chunk(k),
                            ptg[:, r * W:(r + 1) * W],
                            start=(k == 0), stop=(k == nkc - 1))
                dn = ppv.tile([128, W], dt.float32, tag="ppv_ps")
                nc.tensor.matmul(dn[0:1, :], ones_sb[:, 0:1],
                                 dacc[:, 0:W], start=True, stop=False)
                nc.tensor.matmul(dn[0:1, :], ones_sb[:, 0:1],
                                 dacc[:, W:2 * W], start=False, stop=True)
                drc = workp.tile([1, W], dt.float32, tag="drc")
                nc.vector.reciprocal_approx_fast(drc[:], dn[0:1, :])
                drb = workp.tile([128, W], dt.float32, tag="drb")
                nc.gpsimd.partition_broadcast(drb[:], drc[:])
                ots = otsp.tile([128, W], dt.bfloat16, tag="ots")
                nc.vector.tensor_mul(ots[:], ot[:], drb[:])
                return ots

            def oproj_qc(j, ots_heads, qc):
                ob = obp.tile([128, H], dt.float32, tag="ob")
                for n in range(H // W):
                    po = pp.tile([128, W], dt.float32, tag="pp_ps")
                    for a in range(2):
                        nc.tensor.matmul(
                            po[:],
                            ots_heads[a][:, qc * 128:(qc + 1) * 128],
                            wo_sb[:, a * H + n * W: a * H + (n + 1) * W],
                            start=(a == 0), stop=(a == 1))
                    nc.vector.tensor_copy(ob[:, n * W:(n + 1) * W], po[:])
                nc.sync.dma_start(
                    out[j * W + qc * 128: j * W + (qc + 1) * 128, :], ob[:])

            # o_proj(j-1) runs as one contiguous block between the two
            # heads of window j: the PE's LDWEIGHTS pipelining only sustains
            # the 216 ns matmul pace for contiguous runs of one weight
            # stream, so fine-grained interleaving is a net loss (measured)
            prev = None
            for j in range(NW):
                o0 = attn_head(0, j)
                if prev is not None:
                    for qc in range(W // 128):
                        oproj_qc(j - 1, prev, qc)
                o1 = attn_head(1, j)
                prev = (o0, o1)
            for qc in range(W // 128):
                oproj_qc(NW - 1, prev, qc)

    nc.compile()
    return nc


def _prep_inputs(x, cos, sin, Wq, bq, Wk, bk, Wv, bv, Wo):
    x = np.asarray(x, dtype=np.float32).reshape(S, H)
    cos = np.asarray(cos, dtype=np.float32).reshape(S, DH)
    sin = np.asarray(sin, dtype=np.float32).reshape(S, DH)

    xtT = x.T.astype(BF16)                       # [H, S]
    # blocked layout: [seq_block, partition, hid_chunk * W] so each block's
    # DMA is one fully-contiguous read
    xtb = np.ascontiguousarray(
        xtT.reshape(HC, 128, NW, W).transpose(2, 1, 0, 3).reshape(NW, 128, HC * W))

    # head-dim permutation: partition 2t <- dim t, partition 2t+1 <- dim t+64
    perm = np.empty(DH, np.int64)
    perm[0::2] = np.arange(64)
    perm[1::2] = np.arange(64) + 64

    cosT = np.ascontiguousarray(cos.T)          # [128, S]
    sinT = np.ascontiguousarray(sin.T)
    cosP = np.ascontiguousarray(cosT[perm]).astype(BF16)
    sinsP = np.empty_like(sinT)
    sinsP[0::2] = -sinT[:64]
    sinsP[1::2] = sinT[:64]
    sinsP = np.ascontiguousarray(sinsP).astype(BF16)

    # causal 0/1 masks for the 4 diagonal-band chunks of each 512-q window
    kk = np.arange(128)[:, None]
    qq = np.arange(W)[None, :]
    msk = np.concatenate(
        [(qq >= kk + 128 * r).astype(np.float32) for r in range(4)],
        axis=1).astype(BF16)

    Wq = np.asarray(Wq, np.float32)
    Wk = np.asarray(Wk, np.float32)
    Wv = np.asarray(Wv, np.float32)
    Wo = np.asarray(Wo, np.float32)
    bq = np.asarray(bq, np.float32)
    bk = np.asarray(bk, np.float32)
    bv = np.asarray(bv, np.float32)

    in_maps = []
    for c in range(N_CORES):
        kv = c // 4
        qtr = c % 4                      # this core's K/V sequence quarter
        # q/k projections get the RoPE head-dim permutation applied to their
        # output columns (and biases); v/o stay in natural order
        wq_c = np.concatenate(
            [Wq[:, (2 * c + a) * DH:(2 * c + a + 1) * DH][:, perm]
             for a in range(2)], axis=1)
        wk_c = Wk[:, kv * DH:(kv + 1) * DH][:, perm]
        wv_c = Wv[:, kv * DH:(kv + 1) * DH]
        wo_c = Wo[2 * c * DH:(2 * c + 2) * DH, :]
        bq_c = np.stack(
            [bq[(2 * c + a) * DH:(2 * c + a + 1) * DH][perm] for a in range(2)],
            axis=1)
        bkv_c = np.stack(
            [bk[kv * DH:(kv + 1) * DH][perm], bv[kv * DH:(kv + 1) * DH]],
            axis=1)
        def wrearr(w):
            c = w.shape[0] // 128
            return np.ascontiguousarray(
                w.reshape(c, 128, -1).transpose(1, 0, 2).reshape(128, -1))

        in_maps.append({
            "xt": xtb,
            "wq": wrearr(wq_c).astype(BF16),
            "wk": wrearr(wk_c).astype(BF16),
            "wv": wrearr(wv_c).astype(BF16),
            "wo": wrearr(wo_c).astype(BF16),
            "bq": np.ascontiguousarray(bq_c).astype(np.float32),
            "bkv": np.ascontiguousarray(bkv_c).astype(np.float32),
            "cost": cosP, "sins": sinsP,
            "msk": msk,
        })
    return in_maps


def _get_nc():
    if "nc" not in _CACHE:
        _CACHE["nc"] = _build()
    return _CACHE["nc"]


def run(trace=False, tmpdir=None, **inputs):
    from concourse.bass_utils import run_bass_kernel_spmd

    nc = _get_nc()
    in_maps = _prep_inputs(**inputs)
    kw = {}
    if trace:
        kw = dict(trace=True, tmpdir=tmpdir)
    res = run_bass_kernel_spmd(nc, in_maps, core_ids=list(range(N_CORES)), **kw)
    acc = np.zeros((S, H), dtype=np.float32)
    for r in res.results:
        acc += r["out"]
    return acc.reshape(1, S, H), res


def kernel(**inputs) -> np.ndarray:
    out, _ = run(**inputs)
    return out


# revision 36
# speedup vs baseline: 1.1649x; 1.0115x over previous
"""GQA attention block (16 q heads / 2 kv heads, RoPE, causal) on 8 TRN2 NeuronCores.

Strategy: tensor-parallel over heads. Each core owns 2 q heads + the matching
kv head (kv heads replicated over 4-core groups), computes its partial o_proj
output over the full sequence, and the host sums the 8 partials. All cores run
the identical graph; only the input *data* differs per core (SPMD-safe).

Dataflow (everything "transposed" so no on-chip transpose of activations is
ever needed):
  - host passes x^T (bf16) pre-blocked per 512-seq window so every DMA is one
    contiguous read; weights are host-rearranged to [128, chunk*cols] likewise
  - projections compute Q^T (2 heads), K^T and V per window with the weight
    chunk stationary and x^T streaming; K/V are computed by every core
    (replicating them via a 4-core AllGather measured slower than recompute)
  - RoPE head-dim is host-permuted so rotate-half partners sit on adjacent
    partitions: the swap is a DVE within-quadrant stream_shuffle; bias adds
    ride a DVE tensor_scalar_add out of PSUM
  - scores are computed transposed: S^T[key, q] = K^T_chunk.T @ Q^T
  - softmax without max-subtraction, shifted: P = exp(s*scale - 6) on ACT,
    written straight to bf16 (|scaled scores| < 6 so this is exact enough and
    the shift cancels in the ratio)
  - causal masking multiplies the diagonal-band chunks with 0/1 masks (DVE)
  - denominator: fp16 accumulation of P^T groups on DVE (fp16 keeps the 2x
    perf mode; bf16 would too but fp16 halves the rounding error), a
    ones-vector matmul partition-reduce, reciprocal_approx_fast, and a
    gpsimd partition_broadcast; the scale folds into the out^T -> SBUF copy
  - PV accumulates out^T[d, q] with V (natural layout, via PE transpose)
    stationary and P^T streaming
  - o_proj uses out^T slices as the stationary operand directly; o_proj for
    window j-1 is interleaved between the two heads of window j to keep PE fed

Measured: 385.8 us HW exec (neuron-profile), rel l2 err 5.3e-3 vs the fp32
reference. PE is the bottleneck (median matmul spacing at the 216 ns N=512
hardware floor; ~33 us total PE idle).
"""

import os
import sys

for _p in ("/opt/trn_rl_repo",):
    if os.path.isdir(_p) and _p not in sys.path:
        sys.path.append(_p)

import numpy as np
import ml_dtypes

FP16 = np.float16
BF16 = ml_dtypes.bfloat16

# ---- problem constants (hardcoded per harness contract) ----
S = 4096          # sequence length
H = 2048          # hidden
DH = 128          # head dim
N_CORES = 8
HC = H // 128     # 16 hidden chunks
W = 512           # q-window width
NW = S // W       # 8 windows
SQ = S // 4       # sequence quarter (per-core K/V share)
SCALE = 1.0 / float(np.sqrt(DH))
EXP_SHIFT = -6.0

_CACHE = {}


def _build():
    import concourse.bacc as bacc
    import concourse.mybir as mybir
    import concourse.tile as tile
    from concourse.masks import make_identity

    dt = mybir.dt
    AF = mybir.ActivationFunctionType

    nc = bacc.Bacc("TRN2", target_bir_lowering=False, debug=False,
                   num_devices=N_CORES)

    xt = nc.dram_tensor("xt", [NW, 128, HC * W], dt.bfloat16, kind="ExternalInput")
    wq = nc.dram_tensor("wq", [128, HC * 2 * DH], dt.bfloat16, kind="ExternalInput")
    wk = nc.dram_tensor("wk", [128, HC * DH], dt.bfloat16, kind="ExternalInput")
    wv = nc.dram_tensor("wv", [128, HC * DH], dt.bfloat16, kind="ExternalInput")
    wo = nc.dram_tensor("wo", [128, 2 * H], dt.bfloat16, kind="ExternalInput")
    bqd = nc.dram_tensor("bq", [128, 2], dt.float32, kind="ExternalInput")
    bkvd = nc.dram_tensor("bkv", [128, 2], dt.float32, kind="ExternalInput")
    cosd = nc.dram_tensor("cost", [128, S], dt.bfloat16, kind="ExternalInput")
    sind = nc.dram_tensor("sins", [128, S], dt.bfloat16, kind="ExternalInput")
    mskd = nc.dram_tensor("msk", [128, 4 * W], dt.bfloat16, kind="ExternalInput")
    out = nc.dram_tensor("out", [S, H], dt.float32, kind="ExternalOutput")

    with tile.TileContext(nc) as tc:
        with (
            tc.tile_pool(name="const", bufs=1) as constp,
            tc.tile_pool(name="xtp", bufs=2) as xtp,
            tc.tile_pool(name="proj", bufs=1) as projp,
            tc.tile_pool(name="ptp", bufs=8) as ptp,
            tc.tile_pool(name="work", bufs=2) as workp,
            tc.tile_pool(name="otsp", bufs=5) as otsp,
            tc.tile_pool(name="obp", bufs=2) as obp,
            tc.tile_pool(name="pp", bufs=2, space="PSUM") as pp,
            tc.tile_pool(name="pqk", bufs=2, space="PSUM") as pqk,
            tc.tile_pool(name="ppv", bufs=2, space="PSUM") as ppv,
        ):
            # ---------- constants into SBUF ----------
            wq_sb = constp.tile([128, HC * 2 * DH], dt.bfloat16, tag="wq")
            wk_sb = constp.tile([128, HC * DH], dt.bfloat16, tag="wk")
            wv_sb = constp.tile([128, HC * DH], dt.bfloat16, tag="wv")
            wo_sb = constp.tile([128, 2 * H], dt.bfloat16, tag="wo")
            bq_sb = constp.tile([128, 2], dt.float32, tag="bq")
            bkv_sb = constp.tile([128, 2], dt.float32, tag="bkv")
            cos_sb = constp.tile([128, S], dt.bfloat16, tag="cos")
            sin_sb = constp.tile([128, S], dt.bfloat16, tag="sin")
            msk_sb = constp.tile([128, 4 * W], dt.bfloat16, tag="msk")
            ones_sb = constp.tile([128, 1], dt.float16, tag="ones")
            ident = constp.tile([128, 128], dt.bfloat16, tag="ident")
            negC = constp.tile([128, 1], dt.float32, tag="negC")

            # first slice covers hid chunks 0-3 so the first projection
            # matmuls only wait on 0.25 MB; the rest lands right behind xb0
            nc.sync.dma_start(wq_sb[:, 0:1024], wq[:, 0:1024])
            nc.gpsimd.dma_start(wk_sb[:], wk[:, :])
            nc.gpsimd.dma_start(wv_sb[:], wv[:, :])
            nc.gpsimd.dma_start(bq_sb[:], bqd[:, :])
            nc.gpsimd.dma_start(bkv_sb[:], bkvd[:, :])
            nc.gpsimd.dma_start(cos_sb[:], cosd[:, :])
            nc.gpsimd.dma_start(sin_sb[:], sind[:, :])
            nc.gpsimd.dma_start(msk_sb[:], mskd[:, :])
            nc.gpsimd.dma_start(wo_sb[:], wo[:, :])
            nc.gpsimd.memset(ones_sb[:], 1.0)
            nc.gpsimd.memset(negC[:], EXP_SHIFT)
            make_identity(nc, ident[:])

            qt_sb = projp.tile([128, 2 * S], dt.bfloat16, tag="qt")
            kt_q = [projp.tile([128, SQ], dt.bfloat16, tag=f"ktq{r}",
                               name=f"ktq{r}") for r in range(4)]
            vn_q = [projp.tile([128, SQ], dt.bfloat16, tag=f"vnq{r}",
                               name=f"vnq{r}") for r in range(4)]

            def kt_chunk(k):
                return kt_q[k // 8][:, (k % 8) * 128:(k % 8 + 1) * 128]

            def vn_chunk(k):
                return vn_q[k // 8][:, (k % 8) * 128:(k % 8 + 1) * 128]

            shuffle_mask = [i ^ 1 for i in range(32)]

            def rope_store(ps, bias, dest_slc, cslc, sslc):
                t0 = workp.tile([128, W], dt.bfloat16, tag="rope0")
                nc.vector.tensor_scalar_add(t0[:], ps[:], bias)
                tsw = workp.tile([128, W], dt.bfloat16, tag="ropesw")
                nc.vector.stream_shuffle(tsw[:], t0[:], mask=shuffle_mask)
                t1 = workp.tile([128, W], dt.bfloat16, tag="rope1")
                nc.vector.tensor_mul(t1[:], t0[:], cslc)
                t2 = workp.tile([128, W], dt.bfloat16, tag="rope2")
                nc.vector.tensor_mul(t2[:], tsw[:], sslc)
                nc.vector.tensor_add(dest_slc, t1[:], t2[:])

            # ---------- phase 1: Q^T (2 heads) + K^T + V projections ----------
            for sb in range(NW):
                xb = xtp.tile([128, HC * W], dt.bfloat16, tag="xtb")
                nc.sync.dma_start(xb[:], xt[sb, :, :])
                if sb == 0:
                    nc.sync.dma_start(wq_sb[:, 1024:], wq[:, 1024:])
                targets = [
                    ("rope", lambda h: wq_sb[:, h * 256:h * 256 + 128],
                     bq_sb[:, 0:1], qt_sb, 0, cos_sb, sin_sb),
                    ("rope", lambda h: wq_sb[:, h * 256 + 128:h * 256 + 256],
                     bq_sb[:, 1:2], qt_sb, S, cos_sb, sin_sb),
                    ("rope", lambda h: wk_sb[:, h * 128:(h + 1) * 128],
                     bkv_sb[:, 0:1], kt_q[sb // 2], -(sb // 2) * 2 * W,
                     cos_sb, sin_sb),
                    ("vnat", lambda h: wv_sb[:, h * 128:(h + 1) * 128],
                     bkv_sb[:, 1:2], vn_q[sb // 2], 0, None, None),
                ]
                for kind, wslc, bias, dest, doff, ctab, stab in targets:
                    ps = pp.tile([128, W], dt.float32, tag="pp_ps")
                    for h in range(HC):
                        nc.tensor.matmul(
                            ps[:], wslc(h), xb[:, h * W:(h + 1) * W],
                            start=(h == 0), stop=(h == HC - 1))
                    if kind == "rope":
                        rope_store(ps, bias,
                                   dest[:, doff + sb * W: doff + (sb + 1) * W],
                                   ctab[:, sb * W:(sb + 1) * W],
                                   stab[:, sb * W:(sb + 1) * W])
                    else:
                        t0 = workp.tile([128, W], dt.bfloat16, tag="vstage")
                        nc.vector.tensor_scalar_add(t0[:], ps[:], bias)
                        for i in range(W // 128):
                            tp = ppv.tile([128, 128], dt.bfloat16, tag="ppv_ps")
                            nc.tensor.transpose(
                                tp[:], t0[:, i * 128:(i + 1) * 128], ident[:])
                            nc.vector.tensor_copy(
                                dest[:, ((sb % 2) * 4 + i) * 128:
                                     ((sb % 2) * 4 + i + 1) * 128],
                                tp[:])

            # ---------- phase 2: attention, with o_proj(j-1) interleaved ----------
            def attn_head(a, j):
                nkc = 4 * j + 4
                qslc = qt_sb[:, a * S + j * W: a * S + (j + 1) * W]
                ot = ppv.tile([128, W], dt.float32, tag="ppv_ps")
                dacc = workp.tile([128, 2 * W], dt.float16, tag="dacc")
                for g in range(nkc // 2):
                    ps = pqk.tile([128, 2 * W], dt.float32, tag="qk_ps")
                    ptg = ptp.tile([128, 2 * W], dt.bfloat16, tag="pt")
                    for r in range(2):
                        k = 2 * g + r
                        nc.tensor.matmul(
                            ps[:, r * W:(r + 1) * W],
                            kt_chunk(k),
                            qslc, start=True, stop=True)
                    if g == nkc // 2 - 1:
                        # last group = diagonal chunks r=2,3: columns
                        # [0:256] / [512:896] are fully causal-masked, so
                        # exp/mask/dacc/PV all skip them (nothing reads the
                        # skipped ptg region; ot cols are owned by earlier
                        # full-width chunks)
                        nc.scalar.activation(ptg[:, 256:512], ps[:, 256:512],
                                             AF.Exp, scale=SCALE, bias=negC[:])
                        nc.scalar.activation(ptg[:, 896:1024], ps[:, 896:1024],
                                             AF.Exp, scale=SCALE, bias=negC[:])
                        nc.vector.tensor_mul(
                            ptg[:, 256:512], ptg[:, 256:512],
                            msk_sb[:, 2 * W + 256:3 * W])
                        nc.vector.tensor_mul(
                            ptg[:, 896:1024], ptg[:, 896:1024],
                            msk_sb[:, 3 * W + 384:4 * W])
                        nc.vector.tensor_add(dacc[:, 256:512],
                                             dacc[:, 256:512],
                                             ptg[:, 256:512])
                        nc.vector.tensor_add(dacc[:, 896:1024],
                                             dacc[:, 896:1024],
                                             ptg[:, 896:1024])
                        nc.tensor.matmul(
                            ot[:, 256:512], vn_chunk(2 * g),
                            ptg[:, 256:512], start=False, stop=False)
                        nc.tensor.matmul(
                            ot[:, 384:512], vn_chunk(2 * g + 1),
                            ptg[:, 896:1024], start=False, stop=True)
                        continue
                    nc.scalar.activation(ptg[:], ps[:], AF.Exp,
                                         scale=SCALE, bias=negC[:])
                    if g == nkc // 2 - 2:
                        nc.vector.tensor_mul(
                            ptg[:], ptg[:], msk_sb[:, 0:2 * W])
                    if g == 0:
                        nc.vector.tensor_copy(dacc[:], ptg[:])
                    else:
                        nc.vector.tensor_add(dacc[:], dacc[:], ptg[:])
                    for r in range(2):
                        k = 2 * g + r
                        nc.tensor.matmul(
                            ot[:], vn_chunk(k),
                            ptg[:, r * W:(r + 1) * W],
                            start=(k == 0), stop=False)
                dn = ppv.tile([128, W], dt.float32, tag="ppv_ps")
                nc.tensor.matmul(dn[0:1, :], ones_sb[:, 0:1],
                                 dacc[:, 0:W], start=True, stop=False)
                nc.tensor.matmul(dn[0:1, :], ones_sb[:, 0:1],
                                 dacc[:, W:2 * W], start=False, stop=True)
                drc = workp.tile([1, W], dt.float32, tag="drc")
                nc.vector.reciprocal_approx_fast(drc[:], dn[0:1, :])
                drb = workp.tile([128, W], dt.float32, tag="drb")
                nc.gpsimd.partition_broadcast(drb[:], drc[:])
                ots = otsp.tile([128, W], dt.bfloat16, tag="ots")
                nc.vector.tensor_mul(ots[:], ot[:], drb[:])
                return ots

            def oproj_qc(j, ots_heads, qc):
                ob = obp.tile([128, H], dt.float32, tag="ob")
                for n in range(H // W):
                    po = pp.tile([128, W], dt.float32, tag="pp_ps")
                    for a in range(2):
                        nc.tensor.matmul(
                            po[:],
                            ots_heads[a][:, qc * 128:(qc + 1) * 128],
                            wo_sb[:, a * H + n * W: a * H + (n + 1) * W],
                            start=(a == 0), stop=(a == 1))
                    nc.vector.tensor_copy(ob[:, n * W:(n + 1) * W], po[:])
                nc.sync.dma_start(
                    out[j * W + qc * 128: j * W + (qc + 1) * 128, :], ob[:])

            # o_proj(j-1) runs as one contiguous block between the two
            # heads of window j: the PE's LDWEIGHTS pipelining only sustains
            # the 216 ns matmul pace for contiguous runs of one weight
            # stream, so fine-grained interleaving is a net loss (measured)
            prev = None
            for j in range(NW):
                o0 = attn_head(0, j)
                if prev is not None:
                    for qc in range(W // 128):
                        oproj_qc(j - 1, prev, qc)
                o1 = attn_head(1, j)
                prev = (o0, o1)
            for qc in range(W // 128):
                oproj_qc(NW - 1, prev, qc)

    nc.compile()
    return nc


def _prep_inputs(x, cos, sin, Wq, bq, Wk, bk, Wv, bv, Wo):
    x = np.asarray(x, dtype=np.float32).reshape(S, H)
    cos = np.asarray(cos, dtype=np.float32).reshape(S, DH)
    sin = np.asarray(sin, dtype=np.float32).reshape(S, DH)

    xtT = x.T.astype(BF16)                       # [H, S]
    # blocked layout: [seq_block, partition, hid_chunk * W] so each block's
    # DMA is one fully-contiguous read
    xtb = np.ascontiguousarray(
        xtT.reshape(HC, 128, NW, W).transpose(2, 1, 0, 3).reshape(NW, 128, HC * W))

    # head-dim permutation: partition 2t <- dim t, partition 2t+1 <- dim t+64
    perm = np.empty(DH, np.int64)
    perm[0::2] = np.arange(64)
    perm[1::2] = np.arange(64) + 64

    cosT = np.ascontiguousarray(cos.T)          # [128, S]
    sinT = np.ascontiguousarray(sin.T)
    cosP = np.ascontiguousarray(cosT[perm]).astype(BF16)
    sinsP = np.empty_like(sinT)
    sinsP[0::2] = -sinT[:64]
    sinsP[1::2] = sinT[:64]
    sinsP = np.ascontiguousarray(sinsP).astype(BF16)

    # causal 0/1 masks for the 4 diagonal-band chunks of each 512-q window
    kk = np.arange(128)[:, None]
    qq = np.arange(W)[None, :]
    msk = np.concatenate(
        [(qq >= kk + 128 * r).astype(np.float32) for r in range(4)],
        axis=1).astype(BF16)

    Wq = np.asarray(Wq, np.float32)
    Wk = np.asarray(Wk, np.float32)
    Wv = np.asarray(Wv, np.float32)
    Wo = np.asarray(Wo, np.float32)
    bq = np.asarray(bq, np.float32)
    bk = np.asarray(bk, np.float32)
    bv = np.asarray(bv, np.float32)

    in_maps = []
    for c in range(N_CORES):
        kv = c // 4
        qtr = c % 4                      # this core's K/V sequence quarter
        # q/k projections get the RoPE head-dim permutation applied to their
        # output columns (and biases); v/o stay in natural order
        wq_c = np.concatenate(
            [Wq[:, (2 * c + a) * DH:(2 * c + a + 1) * DH][:, perm]
             for a in range(2)], axis=1)
        wk_c = Wk[:, kv * DH:(kv + 1) * DH][:, perm]
        wv_c = Wv[:, kv * DH:(kv + 1) * DH]
        wo_c = Wo[2 * c * DH:(2 * c + 2) * DH, :]
        bq_c = np.stack(
            [bq[(2 * c + a) * DH:(2 * c + a + 1) * DH][perm] for a in range(2)],
            axis=1)
        bkv_c = np.stack(
            [bk[kv * DH:(kv + 1) * DH][perm], bv[kv * DH:(kv + 1) * DH]],
            axis=1)
        def wrearr(w):
            c = w.shape[0] // 128
            return np.ascontiguousarray(
                w.reshape(c, 128, -1).transpose(1, 0, 2).reshape(128, -1))

        in_maps.append({
            "xt": xtb,
            "wq": wrearr(wq_c).astype(BF16),
            "wk": wrearr(wk_c).astype(BF16),
            "wv": wrearr(wv_c).astype(BF16),
            "wo": wrearr(wo_c).astype(BF16),
            "bq": np.ascontiguousarray(bq_c).astype(np.float32),
            "bkv": np.ascontiguousarray(bkv_c).astype(np.float32),
            "cost": cosP, "sins": sinsP,
            "msk": msk,
        })
    return in_maps


def _get_nc():
    if "nc" not in _CACHE:
        _CACHE["nc"] = _build()
    return _CACHE["nc"]


def run(trace=False, tmpdir=None, **inputs):
    from concourse.bass_utils import run_bass_kernel_spmd

    nc = _get_nc()
    in_maps = _prep_inputs(**inputs)
    kw = {}
    if trace:
        kw = dict(trace=True, tmpdir=tmpdir)
    res = run_bass_kernel_spmd(nc, in_maps, core_ids=list(range(N_CORES)), **kw)
    acc = np.zeros((S, H), dtype=np.float32)
    for r in res.results:
        acc += r["out"]
    return acc.reshape(1, S, H), res


def kernel(**inputs) -> np.ndarray:
    out, _ = run(**inputs)
    return out
